# revision 4
# baseline (speedup 1.0000x reference)
"""Trainium2 Bass kernel for nn_ExpertsLinear (weighted mixture of 8 experts).

    y[b, o] = sum_e weights[b, e] * (x @ W[e] + b[e])[b, o]

Full shapes: x [65536, 512] f32, weights [65536, 8] f32,
W [8, 512, 512] f32, b [8, 1, 512] f32 -> y [65536, 512] f32.

Sharding: data-parallel over batch across 8 NeuronCores (8192 rows each);
W replicated. The bias term (always zero in this problem's inputs) is
applied host-side only if nonzero.

Per-core kernel, per 128-row batch tile:
  - DMA x tile f32 -> SBUF, cast to fp16 (DVE)
  - transpose to xT [128 feat, 4, 128 b] via SBUF->SBUF DMA transpose
  - per expert e: z_e = sum_fc xT[:, fc, :].T @ W16[e, fc]  (PSUM fp32,
    one bank per expert, fp16 inputs)
  - combine y = sum_e weights[:, e] * z_e with the scale-muls and add-tree
    split across ScalarE/VectorE/GpSimdE so no single engine outruns PE.
"""

import numpy as np

P = 128
D = 512
E = 8
FC = D // P
N_CORES = 8
B_FULL = 65536
B_LOC = B_FULL // N_CORES

_COMPILED = {}


def _build_nc(transpose_mode="dma"):
    import concourse.bacc as bacc
    import concourse.mybir as mybir
    import concourse.tile as tile
    from concourse.masks import make_identity

    F32 = mybir.dt.float32
    F16 = mybir.dt.float16

    nc = bacc.Bacc(
        "TRN2",
        target_bir_lowering=False,
        debug=False,
        enable_asserts=False,
        num_devices=N_CORES,
    )
    x_d = nc.dram_tensor("x", [B_LOC, D], F32, kind="ExternalInput").ap()
    w_d = nc.dram_tensor("weights", [B_LOC, E], F32, kind="ExternalInput").ap()
    W_d = nc.dram_tensor("W", [E, D, D], F32, kind="ExternalInput").ap()
    y_d = nc.dram_tensor("y", [B_LOC, D], F32, kind="ExternalOutput").ap()

    nbt = B_LOC // P

    with tile.TileContext(nc) as tc:
        with (
            tc.tile_pool(name="const", bufs=1) as const_pool,
            tc.tile_pool(name="xf32", bufs=3) as xf_pool,
            tc.tile_pool(name="xh16", bufs=3) as xh_pool,
            tc.tile_pool(name="xT16", bufs=3) as xT_pool,
            tc.tile_pool(name="zpsum", bufs=8, space="PSUM") as z_pool,
            tc.tile_pool(name="tmul", bufs=2) as t_pool,
            tc.tile_pool(name="yout", bufs=3) as y_pool,
        ):
            # Resident expert weights, cast f32->fp16 by SWDGE on the way in.
            W_sb = const_pool.tile([P, E, FC, D], F16, name="W_sb")
            for e in range(E):
                for fc in range(FC):
                    nc.gpsimd.dma_start(
                        out=W_sb[:, e, fc, :],
                        in_=W_d[e, fc * P : (fc + 1) * P, :],
                    )

            # Resident gate weights: w_sb[p, t, e] = weights[t*128+p, e]
            w_sb = const_pool.tile([P, nbt, E], F32, name="w_sb")
            nc.sync.dma_start(out=w_sb[:], in_=w_d.rearrange("(t p) e -> p t e", p=P))

            if transpose_mode == "pe":
                ident = const_pool.tile([P, P], F16, name="ident")
                make_identity(nc, ident)

            for bt in range(nbt):
                xf = xf_pool.tile([P, D], F32, name="xf")
                nc.sync.dma_start(out=xf[:], in_=x_d[bt * P : (bt + 1) * P, :])
                xh = xh_pool.tile([P, D], F16, name="xh")
                nc.vector.tensor_copy(out=xh[:], in_=xf[:])

                xT = xT_pool.tile([P, FC, P], F16, name="xT")
                if transpose_mode == "pe":
                    with tc.tile_pool(name="tpsum", bufs=1, space="PSUM") as tp_pool:
                        tp = tp_pool.tile([P, FC, P], F16, name="tp")
                        for fc in range(FC):
                            nc.tensor.transpose(
                                tp[:, fc, :], xh[:, fc * P : (fc + 1) * P], ident[:]
                            )
                        nc.vector.tensor_copy(out=xT[:], in_=tp[:])
                else:
                    nc.sync.dma_start_transpose(xT[:], xh[:])

                # z_e accumulates over fc in its own PSUM bank, fp16 inputs.
                z_tiles = [None] * E
                for half in range(2):
                    for fc in range(FC):
                        lhsT = xT[:, fc, :]
                        for ei in range(4):
                            e = half * 4 + ei
                            if fc == 0:
                                z_tiles[e] = z_pool.tile([P, D], F32, name="z")
                            nc.tensor.matmul(
                                z_tiles[e][:],
                                lhsT=lhsT,
                                rhs=W_sb[:, e, fc, :],
                                start=(fc == 0),
                                stop=(fc == FC - 1),
                            )

                # Combine: y = sum_e w[:, e] * z_e.
                # muls: ACT (e0-4) + DVE (e5-7); add tree: GpSimd 3, DVE 4.
                w_col = lambda e: w_sb[:, bt, e : e + 1]
                t = [
                    t_pool.tile([P, D], F32, name=f"t{i}", tag=f"t{i}")
                    for i in range(8)
                ]
                for e in range(5):
                    nc.scalar.mul(t[e][:], z_tiles[e][:], w_col(e))
                for e in range(5, 8):
                    nc.vector.tensor_scalar_mul(t[e][:], z_tiles[e][:], w_col(e))

                p01 = t_pool.tile([P, D], F32, name="p01", tag="p01")
                nc.gpsimd.tensor_add(out=p01[:], in0=t[0][:], in1=t[1][:])
                p23 = t_pool.tile([P, D], F32, name="p23", tag="p23")
                nc.gpsimd.tensor_add(out=p23[:], in0=t[2][:], in1=t[3][:])
                p45 = t_pool.tile([P, D], F32, name="p45", tag="p45")
                nc.vector.tensor_add(out=p45[:], in0=t[4][:], in1=t[5][:])
                p67 = t_pool.tile([P, D], F32, name="p67", tag="p67")
                nc.vector.tensor_add(out=p67[:], in0=t[6][:], in1=t[7][:])
                q0 = t_pool.tile([P, D], F32, name="q0", tag="q0")
                nc.gpsimd.tensor_add(out=q0[:], in0=p01[:], in1=p23[:])
                q1 = t_pool.tile([P, D], F32, name="q1", tag="q1")
                nc.vector.tensor_add(out=q1[:], in0=p45[:], in1=p67[:])
                y_t = y_pool.tile([P, D], F32, name="y_t")
                nc.vector.tensor_add(out=y_t[:], in0=q0[:], in1=q1[:])

                nc.sync.dma_start(out=y_d[bt * P : (bt + 1) * P, :], in_=y_t[:])

    nc.compile()
    return nc


def _get_nc():
    if "nc" not in _COMPILED:
        _COMPILED["nc"] = _build_nc()
    return _COMPILED["nc"]


def kernel(x, weights, W, b):
    from concourse.bass_utils import run_bass_kernel_spmd

    x = np.ascontiguousarray(np.asarray(x, dtype=np.float32))
    weights = np.ascontiguousarray(np.asarray(weights, dtype=np.float32))
    W_np = np.ascontiguousarray(np.asarray(W, dtype=np.float32))
    b_np = np.asarray(b, dtype=np.float32)

    nc = _get_nc()

    xs = x.reshape(N_CORES, B_LOC, D)
    ws = weights.reshape(N_CORES, B_LOC, E)
    in_maps = [
        {"x": xs[c], "weights": ws[c], "W": W_np} for c in range(N_CORES)
    ]
    res = run_bass_kernel_spmd(nc, in_maps, core_ids=list(range(N_CORES)))
    y = np.concatenate([res.results[c]["y"] for c in range(N_CORES)], axis=0)

    # Bias term (zero for this problem's inputs; handled host-side for
    # exactness if ever nonzero).
    if np.any(b_np):
        y = y + weights @ b_np[:, 0, :]

    return y.astype(np.float32)


# revision 5
# speedup vs baseline: 1.1511x; 1.1511x over previous
"""Trainium2 Bass kernel for nn_ExpertsLinear (weighted mixture of 8 experts).

    y[b, o] = sum_e weights[b, e] * (x @ W[e] + b[e])[b, o]

Full shapes: x [65536, 512] f32, weights [65536, 8] f32,
W [8, 512, 512] f32, b [8, 1, 512] f32 -> y [65536, 512] f32.

Sharding: data-parallel over batch across 8 NeuronCores (8192 rows each);
W replicated. The bias term (always zero in this problem's inputs) is
applied host-side only if nonzero.

Per-core kernel, per 128-row batch tile (bt):
  - x tile loaded via SWDGE cast-DMA straight to fp16 SBUF
  - transposed to xT [128 feat, 4, 128 b] by SBUF->SBUF DMA transpose
  - experts grouped 4+4 into two 4-bank PSUM tiles zA/zB; 32 fp16 matmuls
    accumulate z_e = sum_fc xT[:, fc, :].T @ W16[e, fc]
  - combine y = sum_e weights[:, e] * z_e: ScalarE scales group A
    (per-partition scale, fp16 out), VectorE scales group B in one batched
    broadcast mul, then a short fp16 add tree on VectorE.
"""

import numpy as np

P = 128
D = 512
E = 8
FC = D // P
N_CORES = 8
B_FULL = 65536
B_LOC = B_FULL // N_CORES

_COMPILED = {}


def _build_nc():
    import concourse.bacc as bacc
    import concourse.mybir as mybir
    import concourse.tile as tile

    F32 = mybir.dt.float32
    F16 = mybir.dt.float16

    nc = bacc.Bacc(
        "TRN2",
        target_bir_lowering=False,
        debug=False,
        enable_asserts=False,
        num_devices=N_CORES,
    )
    x_d = nc.dram_tensor("x", [B_LOC, D], F32, kind="ExternalInput").ap()
    w_d = nc.dram_tensor("weights", [B_LOC, E], F32, kind="ExternalInput").ap()
    W_d = nc.dram_tensor("W", [E, D, D], F32, kind="ExternalInput").ap()
    y_d = nc.dram_tensor("y", [B_LOC, D], F32, kind="ExternalOutput").ap()

    nbt = B_LOC // P
    HOIST = 3  # x tiles loaded ahead of the W weights on the gpsimd queue

    with tile.TileContext(nc) as tc:
        with (
            tc.tile_pool(name="const", bufs=1) as const_pool,
            tc.tile_pool(name="xh16", bufs=3) as xh_pool,
            tc.tile_pool(name="xT16", bufs=3) as xT_pool,
            tc.tile_pool(name="zpsum", bufs=2, space="PSUM") as z_pool,
            tc.tile_pool(name="tmul", bufs=2) as t_pool,
            tc.tile_pool(name="yout", bufs=3) as y_pool,
        ):
            def load_x(bt):
                xh = xh_pool.tile([P, D], F16, name="xh", tag="xh")
                nc.gpsimd.dma_start(out=xh[:], in_=x_d[bt * P : (bt + 1) * P, :])
                xT = xT_pool.tile([P, FC, P], F16, name="xT", tag="xT")
                nc.sync.dma_start_transpose(xT[:], xh[:])
                return xT

            # Head: first few x tiles ahead of the 8.4MB W load.
            xT_pending = {bt: load_x(bt) for bt in range(min(HOIST, nbt))}

            # Resident expert weights, cast f32->fp16 by SWDGE on the way in.
            W_sb = const_pool.tile([P, E, FC, D], F16, name="W_sb")
            for e in range(E):
                for fc in range(FC):
                    nc.gpsimd.dma_start(
                        out=W_sb[:, e, fc, :],
                        in_=W_d[e, fc * P : (fc + 1) * P, :],
                    )

            # Resident gate weights: w_sb[p, t, e] = weights[t*128+p, e]
            w_sb = const_pool.tile([P, nbt, E], F32, name="w_sb")
            nc.sync.dma_start(out=w_sb[:], in_=w_d.rearrange("(t p) e -> p t e", p=P))

            for bt in range(nbt):
                xT = xT_pending.pop(bt) if bt in xT_pending else load_x(bt)

                # Two expert groups of 4, each one 4-bank PSUM tile.
                zg = [None, None]
                for half in range(2):
                    zg[half] = z_pool.tile([P, 4, D], F32, name="zg", tag="zg")
                    for fc in range(FC):
                        lhsT = xT[:, fc, :]
                        for ei in range(4):
                            nc.tensor.matmul(
                                zg[half][:, ei, :],
                                lhsT=lhsT,
                                rhs=W_sb[:, half * 4 + ei, fc, :],
                                start=(fc == 0),
                                stop=(fc == FC - 1),
                            )

                # Combine: y = sum_e w[:, e] * z_e
                tA = t_pool.tile([P, 4, D], F16, name="tA", tag="tA")
                for ei in range(4):
                    nc.scalar.mul(
                        tA[:, ei, :], zg[0][:, ei, :], w_sb[:, bt, ei : ei + 1]
                    )
                tB = t_pool.tile([P, 4, D], F16, name="tB", tag="tB")
                wB = w_sb[:, bt, 4:8, None].to_broadcast([P, 4, D])
                nc.vector.tensor_mul(out=tB[:], in0=zg[1][:], in1=wB)

                s = t_pool.tile([P, 4, D], F16, name="s", tag="s")
                nc.vector.tensor_add(out=s[:], in0=tA[:], in1=tB[:])
                u = t_pool.tile([P, 2, D], F16, name="u", tag="u")
                nc.vector.tensor_add(out=u[:], in0=s[:, 0:2, :], in1=s[:, 2:4, :])
                y_t = y_pool.tile([P, D], F32, name="y_t")
                nc.vector.tensor_add(out=y_t[:], in0=u[:, 0, :], in1=u[:, 1, :])

                nc.sync.dma_start(out=y_d[bt * P : (bt + 1) * P, :], in_=y_t[:])

    nc.compile()
    return nc


def _get_nc():
    if "nc" not in _COMPILED:
        _COMPILED["nc"] = _build_nc()
    return _COMPILED["nc"]


def kernel(x, weights, W, b):
    from concourse.bass_utils import run_bass_kernel_spmd

    x = np.ascontiguousarray(np.asarray(x, dtype=np.float32))
    weights = np.ascontiguousarray(np.asarray(weights, dtype=np.float32))
    W_np = np.ascontiguousarray(np.asarray(W, dtype=np.float32))
    b_np = np.asarray(b, dtype=np.float32)

    nc = _get_nc()

    xs = x.reshape(N_CORES, B_LOC, D)
    ws = weights.reshape(N_CORES, B_LOC, E)
    in_maps = [
        {"x": xs[c], "weights": ws[c], "W": W_np} for c in range(N_CORES)
    ]
    res = run_bass_kernel_spmd(nc, in_maps, core_ids=list(range(N_CORES)))
    y = np.concatenate([res.results[c]["y"] for c in range(N_CORES)], axis=0)

    # Bias term (zero for this problem's inputs; handled host-side for
    # exactness if ever nonzero).
    if np.any(b_np):
        y = y + weights @ b_np[:, 0, :]

    return y.astype(np.float32)
